# revision 12
# baseline (speedup 1.0000x reference)
"""GATv2 (2-layer, GAT_56727928046275) on 8 TRN2 NeuronCores.

Strategy:
  - Nodes rebalanced (host LPT) into 8 cores x 49 windows of 128 dst
    nodes so per-(core,window) edge counts are nearly equal; edges
    partitioned by dst so per-node softmax + scatter-add stay local.
  - xl tables in DRAM hold |att|-prescaled, sign-permuted features plus
    precomputed "linear logit" columns (lrelu(s)=0.6s+0.4|s| so the
    per-edge logit = 0.6*(Ls+Rd) + 0.4*(sum|.|pos - sum|.|neg)); per-edge
    rows fetched with gpsimd dma_gather (512B rows for layer 0).
  - Scatter indicator generated on-chip (batched DVE is_equal against an
    iota row); the xr-expansion indicator is streamed from DRAM.
  - Layer-0 xl table computed fully on every core (no AllGather); table
    split in two DRAM tensors so block-0 gathers overlap block-1 writes.
  - BatchNorm stats via masked ones-matmul column sums + AllReduce;
    hidden activations stay resident in SBUF between phases.
"""

import numpy as np
import ml_dtypes

import concourse.bass as bass
import concourse.mybir as mybir
import concourse.tile as tile
from concourse.bacc import Bacc
from concourse.bass_utils import run_bass_kernel_spmd

# The ucode's 256B elem_size restriction applies only to transpose-mode
# gathers; relax the bass-level assert so non-transpose gathers can move
# sub-row payloads (row stride stays 256B-aligned via elem_step).
import inspect as _insp
import textwrap as _tw
_src = _tw.dedent(_insp.getsource(bass.BassGpSimd.dma_gather))
_src = _src.replace("elem_size_bytes > 0 and elem_size_bytes % 256 == 0",
                    "elem_size_bytes > 0")
_ns = dict(bass.__dict__)
exec(_src, _ns)
bass.BassGpSimd.dma_gather = _ns["dma_gather"]

BF = mybir.dt.bfloat16
F32 = mybir.dt.float32
I16 = mybir.dt.int16
OP = mybir.AluOpType
AF = mybir.ActivationFunctionType
bf16 = ml_dtypes.bfloat16

NEG_SLOPE = 0.2
BN_EPS = 1e-5


# ---------------------------------------------------------------- host prep

def _wrap16(a):
    """idx array [n] (n % 16 == 0) -> [128, n//16] int16 (16-wrap, x8 rep)."""
    a = np.asarray(a, np.int16).reshape(-1, 16).T  # [16, n/16]
    return np.tile(a, (8, 1))


def _rep(v, dt=np.float32):
    v = np.asarray(v, dt).reshape(1, -1)
    return np.ascontiguousarray(np.broadcast_to(v, (128, v.shape[1])))


def _sign_perm(att):
    """Per-head permutation putting att>=0 channels first.
    att: [H, C]. Returns perm [H*C] (flat), npos [H]."""
    H, C = att.shape
    perm = np.zeros(H * C, np.int64)
    npos = np.zeros(H, np.int64)
    for h in range(H):
        a = att[h]
        pos = np.nonzero(a >= 0)[0]
        neg = np.nonzero(a < 0)[0]
        perm[h * C:(h + 1) * C] = h * C + np.concatenate([pos, neg])
        npos[h] = len(pos)
    return perm, npos


def _balance_nodes(deg, n_cores, W):
    """LPT-assign nodes to n_cores*W bins of <=128 nodes, balancing the
    per-bin in-degree sums. Returns rowid[n] (table row per node)."""
    import heapq
    N = deg.shape[0]
    nbins = n_cores * W
    order = np.argsort(-deg, kind="stable")
    heap = [(0, b) for b in range(nbins)]
    heapq.heapify(heap)
    counts = np.zeros(nbins, np.int64)
    rowid = np.zeros(N, np.int64)
    spill = []
    for n in order:
        while True:
            load, b = heapq.heappop(heap)
            if counts[b] < 128:
                break
            spill.append((load, b))
        rowid[n] = b * 128 + counts[b]
        counts[b] += 1
        heapq.heappush(heap, (load + int(deg[n]), b))
    return rowid


def preprocess(x, edge_index, params, n_cores=8):
    """Build per-core input maps + program metadata."""
    N, IN = x.shape
    SLICE = N // n_cores
    SP = ((SLICE + 127) // 128) * 128          # padded slice rows (6272)
    W = SP // 128                              # windows per core (49)
    BLK = (n_cores // 2) * SP                  # int16 table split row (25088)

    src = np.concatenate([edge_index[0], np.arange(N, dtype=np.int64)])
    dst = np.concatenate([edge_index[1], np.arange(N, dtype=np.int64)])

    deg = np.bincount(dst, minlength=N)
    rowid = _balance_nodes(deg, n_cores, W)    # node -> table row
    row = rowid[src]                           # per-edge src table row
    drow = rowid[dst]
    core = drow // SP
    dloc = drow % SP                           # dst row within its core

    # per-core, per-window, per-block edge lists
    ecnt0 = np.zeros((n_cores, W), np.int64)
    ecnt1 = np.zeros((n_cores, W), np.int64)
    lists = [[None] * W for _ in range(n_cores)]
    for k in range(n_cores):
        m = core == k
        r, d = row[m], dloc[m]
        win = d // 128
        blk = (r >= BLK).astype(np.int64)
        order = np.lexsort((blk, win))
        r, d, win, blk = r[order], d[order], win[order], blk[order]
        for w in range(W):
            wm = win == w
            rw, dw, bw = r[wm], d[wm], blk[wm]
            b0 = bw == 0
            lists[k][w] = (rw[b0], dw[b0], rw[~b0], dw[~b0])
            ecnt0[k][w] = int(b0.sum())
            ecnt1[k][w] = int((~b0).sum())

    G0 = np.maximum(1, (ecnt0.max(0) + 127) // 128)   # [W]
    G1 = np.maximum(1, (ecnt1.max(0) + 127) // 128)
    GW = G0 + G1
    NG = int(GW.sum())

    # padded per-core arrays
    src_cols = int((GW * 8).sum())
    srcw = np.zeros((n_cores, 128, src_cols), np.int16)
    drel = np.full((n_cores, 128, NG), -1.0, np.float32)
    swin = np.zeros((n_cores, 128, NG * 128), bf16)
    for k in range(n_cores):
        sc = 0
        gc = 0
        for w in range(W):
            r0, d0, r1, d1 = lists[k][w]
            n0, n1 = len(r0), len(r1)
            s0 = np.zeros(int(G0[w]) * 128, np.int64)
            s0[:n0] = r0
            s1 = np.zeros(int(G1[w]) * 128, np.int64)
            s1[:n1] = r1 - BLK
            rl = np.full(int(GW[w]) * 128, -1.0, np.float32)
            rl[:n0] = (d0 - w * 128).astype(np.float32)
            rl[int(G0[w]) * 128:int(G0[w]) * 128 + n1] = (
                d1 - w * 128).astype(np.float32)
            srcw[k][:, sc:sc + int(G0[w]) * 8] = _wrap16(s0)
            srcw[k][:, sc + int(G0[w]) * 8:sc + int(GW[w]) * 8] = _wrap16(s1)
            drel[k][:, gc:gc + int(GW[w])] = rl.reshape(int(GW[w]), 128).T
            # node-major indicator for xr expansion: [n, g*128+e]
            sw = (np.arange(128)[:, None] == rl[None, :]).astype(bf16)
            swin[k][:, gc * 128:(gc + int(GW[w])) * 128] = sw
            sc += int(GW[w]) * 8
            gc += int(GW[w])

    p = params
    H, C = p["att0"].shape
    OUTC = p["att1"].shape[1]
    perm0, npos0 = _sign_perm(np.asarray(p["att0"]))
    perm1, npos1 = _sign_perm(np.asarray(p["att1"]))
    aab0 = np.abs(np.asarray(p["att0"]).reshape(-1)[perm0])   # [192]
    aab1 = np.abs(np.asarray(p["att1"]).reshape(-1)[perm1])   # [64]

    def sign_vec(npos, nh, Ch):
        s = np.zeros(nh * Ch, np.float32)
        for h in range(nh):
            s[h * Ch:h * Ch + int(npos[h])] = 1.0
            s[h * Ch + int(npos[h]):(h + 1) * Ch] = -1.0
        return s

    sgn0 = sign_vec(npos0, H, C)
    sgn1 = sign_vec(npos1, 1, OUTC)

    def prep_w(Wm, b, perm, aab, sgn, nh, Ch):
        """Permute+prescale weight; append nh linear-logit columns."""
        Wp = np.asarray(Wm, np.float64)[:, perm] * aab[None, :]
        bp = np.asarray(b, np.float64)[perm] * aab
        La = np.stack([(Wp[:, h * Ch:(h + 1) * Ch]
                        * sgn[h * Ch:(h + 1) * Ch][None, :]).sum(1)
                       for h in range(nh)], 1)      # [in, nh]
        Lb = np.array([(bp[h * Ch:(h + 1) * Ch]
                        * sgn[h * Ch:(h + 1) * Ch]).sum()
                       for h in range(nh)])
        return (np.concatenate([Wp, La], 1).astype(np.float32),
                np.concatenate([bp, Lb]).astype(np.float32))

    Wl0p, bl0p = prep_w(p["Wl0"], p["bl0"], perm0, aab0, sgn0, H, C)
    Wr0p, br0p = prep_w(p["Wr0"], p["br0"], perm0, aab0, sgn0, H, C)
    Wl1p, bl1p = prep_w(np.asarray(p["Wl1"])[perm0, :], p["bl1"],
                        perm1, aab1, sgn1, 1, OUTC)
    Wr1p, br1p = prep_w(np.asarray(p["Wr1"])[perm0, :], p["br1"],
                        perm1, aab1, sgn1, 1, OUTC)

    FEAT0 = H * C            # 192
    NC0 = FEAT0 + H          # 195 computed cols layer 0
    NC1 = OUTC + 1           # 65 computed cols layer 1
    TBL0 = 256               # bf16 cols -> 512B rows
    TBL1 = 128               # bf16 cols -> 256B rows
    ELEM1 = 66               # gathered cols layer 1 (>=65, mult of 2)

    shared = {
        "Wl0p": Wl0p.astype(bf16), "Wr0p": Wr0p.astype(bf16),
        "BL0": _rep(bl0p, bf16), "BR0": _rep(br0p, bf16),
        "Wl1a": Wl1p[:128].astype(bf16), "Wl1b": Wl1p[128:].astype(bf16),
        "Wr1a": Wr1p[:128].astype(bf16), "Wr1b": Wr1p[128:].astype(bf16),
        "BL1": _rep(bl1p), "BR1": _rep(br1p),
        "RA0": _rep(1.0 / aab0), "RA1": _rep(1.0 / aab1),
        "BIAS0": _rep(np.asarray(p["bias0"])[perm0]),
        "BIAS1": _rep(np.asarray(p["bias1"])[perm1]),
        "gamma0": np.asarray(p["gamma0"])[perm0].reshape(1, -1).astype(
            np.float32),
        "beta0": np.asarray(p["beta0"])[perm0].reshape(1, -1).astype(
            np.float32),
        "gamma1": np.asarray(p["gamma1"])[perm1].reshape(1, -1).astype(
            np.float32),
        "beta1": np.asarray(p["beta1"])[perm1].reshape(1, -1).astype(
            np.float32),
        "IDENT": np.eye(128, dtype=bf16),
        "ONE1": np.ones((1, 128), np.float32),
    }
    shared["IOTA"] = np.tile(np.arange(128, dtype=bf16).reshape(1, -1),
                             (128, 1))

    xt = np.asarray(x).T  # [IN, N]
    xfull = np.zeros((IN, n_cores * SP), np.float32)
    xfull[:, :] = 0.0
    cols = rowid  # node n -> column rowid[n]
    xfull[:, cols] = xt
    xfull = xfull.astype(bf16)

    in_maps = []
    for k in range(n_cores):
        # per-core validity mask [128, W] (1 for slots holding real nodes)
        msk = np.zeros(SP, np.float32)
        kcols = (rowid >= k * SP) & (rowid < (k + 1) * SP)
        msk[rowid[kcols] - k * SP] = 1.0
        m = {"xT": xfull,
             "xTk": np.ascontiguousarray(xfull[:, k * SP:(k + 1) * SP]),
             "srcw": srcw[k], "swin": swin[k],
             "drel": drel[k].astype(bf16),
             "SMASK": np.ascontiguousarray(
                 msk.reshape(W, 128).T.astype(bf16))}
        for kk, v in shared.items():
            m[kk] = np.ascontiguousarray(v)
        in_maps.append(m)

    meta = dict(N=N, IN=IN, SLICE=SLICE, SP=SP, W=W, BLK=BLK,
                G0=[int(v) for v in G0], G1=[int(v) for v in G1],
                GW=[int(v) for v in GW], NG=NG, src_cols=src_cols,
                H=H, C=C, FEAT0=FEAT0, OUTC=OUTC, NC0=NC0, NC1=NC1,
                TBL0=TBL0, TBL1=TBL1, ELEM1=ELEM1, Gmax=int(GW.max()),
                npos0=[int(v) for v in npos0], npos1=[int(v) for v in npos1],
                n_cores=n_cores)
    return in_maps, meta, rowid, perm1


# ---------------------------------------------------------------- program

def build_program(meta):
    n_cores = meta["n_cores"]
    IN, SP, W = meta["IN"], meta["SP"], meta["W"]
    G0, G1, GW, NG = meta["G0"], meta["G1"], meta["GW"], meta["NG"]
    H, C, FEAT0 = meta["H"], meta["C"], meta["FEAT0"]
    OUTC = meta["OUTC"]
    NC0, NC1 = meta["NC0"], meta["NC1"]
    TBL0, TBL1, ELEM1 = meta["TBL0"], meta["TBL1"], meta["ELEM1"]
    BLKROW = (n_cores // 2) * SP
    HBLK = BLKROW // 128                       # 128-row tiles per table half
    NROW = n_cores * SP
    cores = list(range(n_cores))
    import os
    GB = int(os.environ.get("K_GB", "8"))      # gather groups per SWDGE op

    nc = Bacc("TRN2", target_bir_lowering=False, debug=False,
              num_devices=n_cores, num_swdge_queues=4)

    def dp(name, shape, dt):
        return nc.declare_dram_parameter(name, list(shape), dt,
                                         isOutput=False)

    xT = dp("xT", [IN, NROW], BF)
    xTk = dp("xTk", [IN, SP], BF)
    srcw = dp("srcw", [128, meta["src_cols"]], I16)
    swin = dp("swin", [128, NG * 128], BF)
    drel = dp("drel", [128, NG], BF)
    SMASK = dp("SMASK", [128, W], BF)
    Wl0p = dp("Wl0p", [IN, NC0], BF)
    Wr0p = dp("Wr0p", [IN, NC0], BF)
    BL0 = dp("BL0", [128, NC0], BF)
    BR0 = dp("BR0", [128, NC0], BF)
    Wl1a = dp("Wl1a", [128, NC1], BF)
    Wl1b = dp("Wl1b", [FEAT0 - 128, NC1], BF)
    Wr1a = dp("Wr1a", [128, NC1], BF)
    Wr1b = dp("Wr1b", [FEAT0 - 128, NC1], BF)
    BL1 = dp("BL1", [128, NC1], F32)
    BR1 = dp("BR1", [128, NC1], F32)
    RA0 = dp("RA0", [128, FEAT0], F32)
    RA1 = dp("RA1", [128, OUTC], F32)
    BIAS0 = dp("BIAS0", [128, FEAT0], F32)
    BIAS1 = dp("BIAS1", [128, OUTC], F32)
    gamma0 = dp("gamma0", [1, FEAT0], F32)
    beta0 = dp("beta0", [1, FEAT0], F32)
    gamma1 = dp("gamma1", [1, OUTC], F32)
    beta1 = dp("beta1", [1, OUTC], F32)
    IOTA = dp("IOTA", [128, 128], BF)
    IDENT = dp("IDENT", [128, 128], BF)
    ONE1 = dp("ONE1", [1, 128], F32)

    out = nc.declare_dram_parameter("out", [SP, OUTC], F32, isOutput=True)

    # internal DRAM
    XL0a = nc.dram_tensor("XL0a", [BLKROW, TBL0], BF)
    XL0b = nc.dram_tensor("XL0b", [NROW - BLKROW, TBL0], BF)
    XR0 = nc.dram_tensor("XR0", [SP, TBL0], BF)
    XL1s = nc.dram_tensor("XL1s", [SP, TBL1], BF)
    XR1 = nc.dram_tensor("XR1", [SP, TBL1], BF)
    XL1 = nc.dram_tensor("XL1", [NROW, TBL1], BF, addr_space="Shared")
    ST0i = nc.dram_tensor("ST0i", [1, 2 * FEAT0], F32)
    ST0o = nc.dram_tensor("ST0o", [1, 2 * FEAT0], F32, addr_space="Shared")
    ST1i = nc.dram_tensor("ST1i", [1, 2 * OUTC], F32)
    ST1o = nc.dram_tensor("ST1o", [1, 2 * OUTC], F32, addr_space="Shared")

    with tile.TileContext(nc) as tc:
        with tc.tile_pool(name="const", bufs=1) as cp, \
             tc.tile_pool(name="hp0p", bufs=1) as hp0p, \
             tc.tile_pool(name="hp1p", bufs=1) as hp1p:
            def ld(par, shape, dt):
                t = cp.tile(list(shape), dt, tag=f"c_{par.name}")
                nc.sync.dma_start(out=t[:], in_=par[:])
                return t

            c_iota = ld(IOTA, [128, 128], BF)
            c_ident = ld(IDENT, [128, 128], BF)
            c_one1 = ld(ONE1, [1, 128], F32)
            c_smask = ld(SMASK, [128, W], BF)
            c_wl0 = ld(Wl0p, [IN, NC0], BF)
            c_wr0 = ld(Wr0p, [IN, NC0], BF)
            c_bl0 = ld(BL0, [128, NC0], BF)
            c_br0 = ld(BR0, [128, NC0], BF)
            c_wl1a = ld(Wl1a, [128, NC1], BF)
            c_wl1b = ld(Wl1b, [FEAT0 - 128, NC1], BF)
            c_wr1a = ld(Wr1a, [128, NC1], BF)
            c_wr1b = ld(Wr1b, [FEAT0 - 128, NC1], BF)
            c_bl1 = ld(BL1, [128, NC1], F32)
            c_br1 = ld(BR1, [128, NC1], F32)
            c_ra0 = ld(RA0, [128, FEAT0], F32)
            c_ra1 = ld(RA1, [128, OUTC], F32)
            c_bias0 = ld(BIAS0, [128, FEAT0], F32)
            c_bias1 = ld(BIAS1, [128, OUTC], F32)
            c_g0 = ld(gamma0, [1, FEAT0], F32)
            c_b0 = ld(beta0, [1, FEAT0], F32)
            c_g1 = ld(gamma1, [1, OUTC], F32)
            c_b1 = ld(beta1, [1, OUTC], F32)
            c_src = ld(srcw, [128, meta["src_cols"]], I16)
            c_drel = ld(drel, [128, NG], BF)

            # ---------------- stage A: full xl0 table + local xr0 ------
            with tc.tile_pool(name="sta", bufs=4) as sa, \
                 tc.tile_pool(name="sta_ps", bufs=4, space="PSUM") as sap:
                def tbl_tile(xsrc, t, wmat, btile, dram, trow, ncols):
                    xt_t = sa.tile([IN, 128], BF, tag="xt")
                    nc.sync.dma_start(out=xt_t[:],
                                      in_=xsrc[:, t * 128:(t + 1) * 128])
                    ps = sap.tile([128, 256], F32, tag="ps")
                    nc.tensor.matmul(ps[:, 0:ncols], xt_t[:], wmat[:],
                                     start=True, stop=True)
                    sb = sa.tile([128, TBL0], BF, tag="sb")
                    nc.vector.tensor_add(sb[:, 0:ncols], ps[:, 0:ncols],
                                         btile[:])
                    nc.sync.dma_start(
                        out=dram[trow * 128:(trow + 1) * 128, :],
                        in_=sb[:])

                for t in range(HBLK):
                    tbl_tile(xT, t, c_wl0, c_bl0, XL0a, t, NC0)
                for t in range(HBLK, NROW // 128):
                    tbl_tile(xT, t, c_wl0, c_bl0, XL0b, t - HBLK, NC0)
                for t in range(W):
                    tbl_tile(xTk, t, c_wr0, c_br0, XR0, t, NC0)

            # ---------------- edge pass helper -------------------------
            def edge_pass(XLa, XLb, XRt, tblc, elem, feat, ncol, nh, npos,
                          ra, biast, hp_pool, st_in):
                """One GATv2 edge pass; returns list of SBUF hp tiles."""
                ssum = cp.tile([1, 2 * feat], F32, tag=f"ssum{feat}")
                hp_tiles = []
                with tc.tile_pool(name="eg", bufs=3) as eg, \
                     tc.tile_pool(name="es", bufs=3) as es, \
                     tc.tile_pool(name="eps", bufs=2, space="PSUM") as eps, \
                     tc.tile_pool(name="xrp", bufs=1, space="PSUM") as xrp, \
                     tc.tile_pool(name="sps", bufs=1, space="PSUM") as sps:
                    sc = 0
                    gc = 0
                    spa = sps.tile([1, feat], F32, tag="spa")
                    spb = sps.tile([1, feat], F32, tag="spb")
                    qrr = [0]

                    def gathers(dst_tile, col0, tbl_ap, idx_col0, n_groups):
                        done = 0
                        while done < n_groups:
                            cnt = min(GB, n_groups - done)
                            nc.gpsimd.dma_gather(
                                dst_tile[:, col0 + done:col0 + done + cnt, :],
                                tbl_ap,
                                c_src[:, idx_col0 + done * 8:
                                      idx_col0 + (done + cnt) * 8],
                                num_idxs=cnt * 128, num_idxs_reg=cnt * 128,
                                elem_size=elem, elem_step=tblc,
                                queue_num=qrr[0])
                            qrr[0] = (qrr[0] + 1) % 4
                            done += cnt

                    for w in range(W):
                        g0, g1, gw = G0[w], G1[w], GW[w]
                        gl = eg.tile([128, gw, elem], BF, tag="gl")
                        gathers(gl, 0, XLa, sc, g0)
                        gathers(gl, g0, XLb, sc + g0 * 8, g1)
                        xw = eg.tile([128, elem], BF, tag="xw")
                        nc.sync.dma_start(
                            out=xw[:],
                            in_=XRt[w * 128:(w + 1) * 128, 0:elem])
                        sw = eg.tile([128, gw, 128], BF, tag="sw")
                        nc.sync.dma_start(
                            out=sw[:],
                            in_=swin[:, gc * 128:(gc + gw) * 128])
                        # on-chip scatter indicator: sal[e, g, d]=(drel==d)
                        sal = es.tile([128, gw, 128], BF, tag="sal")
                        nc.vector.tensor_tensor(
                            out=sal[:],
                            in0=c_drel[:, gc:gc + gw].unsqueeze(2)
                                .broadcast_to([128, gw, 128]),
                            in1=c_iota[:].unsqueeze(1)
                                .broadcast_to([128, gw, 128]),
                            op=OP.is_equal)
                        z = es.tile([128, gw, ncol], BF, tag="z")
                        for b0 in range(0, gw, 4):
                            nb = min(4, gw - b0)
                            xre = xrp.tile([128, 4, 512], F32, tag="xre")
                            for j in range(nb):
                                nc.tensor.matmul(
                                    xre[:, j, 0:ncol], sw[:, b0 + j, :],
                                    xw[:, 0:ncol], start=True, stop=True)
                            nc.vector.tensor_add(
                                z[:, b0:b0 + nb, :],
                                gl[:, b0:b0 + nb, 0:ncol],
                                xre[:, 0:nb, 0:ncol])
                        lp = es.tile([128, gw, nh], BF, tag="lp")
                        ln = es.tile([128, gw, nh], BF, tag="ln")
                        with nc.allow_low_precision(
                                reason="bf16 logit partial sums (<=64 "
                                       "terms, |logit|<~2)"):
                            for h in range(nh):
                                k = npos[h]
                                if k == 0:
                                    nc.vector.memset(lp[:, :, h], 0.0)
                                else:
                                    nc.vector.tensor_reduce(
                                        lp[:, :, h],
                                        z[:, :, h * C:h * C + k],
                                        axis=mybir.AxisListType.X, op=OP.add,
                                        apply_absolute_value=True)
                                if k == C:
                                    nc.vector.memset(ln[:, :, h], 0.0)
                                else:
                                    nc.vector.tensor_reduce(
                                        ln[:, :, h],
                                        z[:, :, h * C + k:(h + 1) * C],
                                        axis=mybir.AxisListType.X, op=OP.add,
                                        apply_absolute_value=True)
                        rhs = es.tile([128, gw, ncol], BF, tag="rhs")
                        lg = es.tile([128, gw, nh], BF, tag="lg")
                        nc.vector.tensor_sub(lg[:], lp[:], ln[:])
                        # logit = 0.6*(Ls+Rd) + 0.4*(lp-ln)
                        nc.vector.scalar_tensor_tensor(
                            lg[:], lg[:], 2.0 / 3.0, z[:, :, feat:feat + nh],
                            op0=OP.mult, op1=OP.add)
                        nc.scalar.activation(
                            rhs[:, :, feat:feat + nh], lg[:], AF.Exp,
                            scale=0.6)
                        pb = rhs[:, :, feat:feat + nh].unsqueeze(3)
                        nc.vector.tensor_mul(
                            rhs[:, :, 0:feat].rearrange(
                                "p g (h c) -> p g h c", h=nh),
                            gl[:, :, 0:feat].rearrange(
                                "p g (h c) -> p g h c", h=nh),
                            pb.broadcast_to([128, gw, nh, C]))
                        ps = eps.tile([128, ncol], F32, tag="acc")
                        for g in range(gw):
                            nc.tensor.matmul(
                                ps[:], sal[:, g, :], rhs[:, g, :],
                                start=(g == 0), stop=(g == gw - 1))
                        # finalize
                        dn = es.tile([128, nh], F32, tag="dn")
                        nc.vector.tensor_scalar_max(
                            dn[:], ps[:, feat:feat + nh], 1e-30)
                        rc = es.tile([128, nh], F32, tag="rc")
                        nc.vector.reciprocal(rc[:], dn[:])
                        hp = hp_pool.tile([128, feat], BF, tag=f"hp{w}")
                        hp_tiles.append(hp)
                        tmp = es.tile([128, feat], F32, tag="tmp")
                        for h in range(nh):
                            nc.vector.scalar_tensor_tensor(
                                tmp[:, h * C:(h + 1) * C],
                                ps[:, h * C:(h + 1) * C],
                                rc[:, h:h + 1],
                                ra[:, h * C:(h + 1) * C],
                                op0=OP.mult, op1=OP.mult)
                        nc.vector.tensor_add(tmp[:], tmp[:], biast[:])
                        nc.scalar.activation(hp[:], tmp[:], AF.Relu)
                        # stats
                        sq = es.tile([128, feat], BF, tag="sq")
                        nc.scalar.activation(sq[:], hp[:], AF.Square)
                        nc.tensor.matmul(spa[:], c_smask[:, w:w + 1], hp[:],
                                         start=(w == 0), stop=(w == W - 1))
                        nc.tensor.matmul(spb[:], c_smask[:, w:w + 1], sq[:],
                                         start=(w == 0), stop=(w == W - 1))
                        sc += gw * 8
                        gc += gw
                    nc.vector.tensor_copy(ssum[:, 0:feat], spa[:])
                    nc.vector.tensor_copy(ssum[:, feat:2 * feat], spb[:])
                nc.sync.dma_start(out=st_in[:, :], in_=ssum[:])
                return hp_tiles

            # ---------------- BN coeff helper --------------------------
            def bn_coeffs(st_out, feat, g_row, b_row, nodes):
                """AllReduced stats -> A_rep/B_rep [128, feat]."""
                st = cp.tile([1, 2 * feat], F32, tag=f"st{feat}")
                nc.sync.dma_start(out=st[:], in_=st_out[:, :])
                mu = cp.tile([1, feat], F32, tag=f"mu{feat}")
                var = cp.tile([1, feat], F32, tag=f"va{feat}")
                nc.vector.tensor_scalar_mul(mu[:], st[:, 0:feat], 1.0 / nodes)
                nc.vector.tensor_scalar_mul(var[:], st[:, feat:2 * feat],
                                            1.0 / nodes)
                t = cp.tile([1, feat], F32, tag=f"t{feat}")
                nc.vector.tensor_mul(t[:], mu[:], mu[:])
                nc.vector.tensor_sub(var[:], var[:], t[:])
                nc.vector.tensor_scalar_add(var[:], var[:], BN_EPS)
                nc.scalar.activation(t[:], var[:], AF.Sqrt)
                nc.vector.reciprocal(t[:], t[:])          # 1/sqrt(var+eps)
                arow = cp.tile([1, feat], F32, tag=f"ar{feat}")
                nc.vector.tensor_mul(arow[:], g_row[:], t[:])
                brow = cp.tile([1, feat], F32, tag=f"br{feat}")
                nc.vector.tensor_mul(t[:], mu[:], arow[:])
                nc.vector.tensor_sub(brow[:], b_row[:], t[:])
                with tc.tile_pool(name="bnp", bufs=2, space="PSUM") as bp:
                    pa = bp.tile([128, feat], F32, tag="pa")
                    nc.tensor.matmul(pa[:], c_one1[:], arow[:],
                                     start=True, stop=True)
                    Ar = cp.tile([128, feat], BF, tag=f"A{feat}")
                    nc.scalar.copy(Ar[:], pa[:])
                    pb2 = bp.tile([128, feat], F32, tag="pb")
                    nc.tensor.matmul(pb2[:], c_one1[:], brow[:],
                                     start=True, stop=True)
                    Br = cp.tile([128, feat], BF, tag=f"B{feat}")
                    nc.scalar.copy(Br[:], pb2[:])
                return Ar, Br

            # ================= layer 0 =================================
            hp0 = edge_pass(XL0a[:, :], XL0b[:, :], XR0, TBL0, TBL0, FEAT0,
                            NC0, H, meta["npos0"], c_ra0, c_bias0, hp0p,
                            ST0i)
            nc.gpsimd.collective_compute(
                "AllReduce", OP.add, replica_groups=[cores],
                ins=[ST0i[:, :]], outs=[ST0o[:, :]])
            A0, B0 = bn_coeffs(ST0o, FEAT0, c_g0, c_b0, meta["N"])

            # pass 2: BN apply + layer-1 tables (hp tiles from SBUF)
            with tc.tile_pool(name="p2", bufs=3) as p2, \
                 tc.tile_pool(name="p2ps", bufs=2, space="PSUM") as pp:
                for w in range(W):
                    hb = p2.tile([128, FEAT0], BF, tag="hb")
                    hp = hp0[w]
                    nc.vector.tensor_mul(hb[:], hp[:], A0[:])
                    nc.vector.tensor_add(hb[:], hb[:], B0[:])
                    pt0 = pp.tile([128, 128], BF, tag="pt0")
                    nc.tensor.transpose(pt0[:], hb[:, 0:128], c_ident[:])
                    pt1 = pp.tile([64, 128], BF, tag="pt1")
                    nc.tensor.transpose(pt1[:], hb[:, 128:192],
                                        c_ident[:])
                    ht0 = p2.tile([128, 128], BF, tag="ht0")
                    nc.scalar.copy(ht0[:], pt0[:])
                    ht1 = p2.tile([64, 128], BF, tag="ht1")
                    nc.scalar.copy(ht1[:], pt1[:])
                    for (wa, wb, btile, dram) in (
                            (c_wl1a, c_wl1b, c_bl1, XL1s),
                            (c_wr1a, c_wr1b, c_br1, XR1)):
                        px = pp.tile([128, NC1], F32, tag="px")
                        nc.tensor.matmul(px[:], ht0[:], wa[:],
                                         start=True, stop=False)
                        nc.tensor.matmul(px[:], ht1[:], wb[:],
                                         start=False, stop=True)
                        xs = p2.tile([128, TBL1], BF, tag="xs")
                        nc.vector.tensor_add(xs[:, 0:NC1], px[:], btile[:])
                        nc.sync.dma_start(
                            out=dram[w * 128:(w + 1) * 128, :],
                            in_=xs[:])

            nc.gpsimd.collective_compute(
                "AllGather", OP.bypass, replica_groups=[cores],
                ins=[XL1s[:, :]], outs=[XL1[:, :]])

            # ================= layer 1 =================================
            hp1 = edge_pass(XL1[0:BLKROW, 0:ELEM1], XL1[BLKROW:, 0:ELEM1],
                            XR1, TBL1, ELEM1, OUTC, NC1, 1, meta["npos1"],
                            c_ra1, c_bias1, hp1p, ST1i)
            nc.gpsimd.collective_compute(
                "AllReduce", OP.add, replica_groups=[cores],
                ins=[ST1i[:, :]], outs=[ST1o[:, :]])
            A1, B1 = bn_coeffs(ST1o, OUTC, c_g1, c_b1, meta["N"])

            with tc.tile_pool(name="p3", bufs=3) as p3:
                for w in range(W):
                    hp = hp1[w]
                    ob = p3.tile([128, OUTC], F32, tag="ob")
                    nc.vector.tensor_mul(ob[:], hp[:], A1[:])
                    nc.vector.tensor_add(ob[:], ob[:], B1[:])
                    nc.sync.dma_start(
                        out=out[w * 128:(w + 1) * 128, :],
                        in_=ob[:])

    nc.compile()
    return nc


# ---------------------------------------------------------------- entry

def kernel(**inputs):
    x = np.asarray(inputs["x"])
    edge_index = np.asarray(inputs["edge_index"])
    params = {k: np.asarray(v) for k, v in inputs.items()
              if k not in ("x", "edge_index")}
    n_cores = 8
    in_maps, meta, rowid, perm1 = preprocess(x, edge_index, params, n_cores)
    nc = build_program(meta)
    import os
    trace = bool(int(os.environ.get("K_TRACE", "0")))
    res = run_bass_kernel_spmd(nc, in_maps, list(range(n_cores)),
                               trace=trace)
    if trace:
        print(f"HW exec time: {res.exec_time_ns} ns", flush=True)
    outs = [res.results[k]["out"] for k in range(n_cores)]
    full = np.concatenate(outs, 0)       # [n_cores*SP, OUTC]
    inv = np.argsort(perm1)
    return np.ascontiguousarray(full[rowid][:, inv]).astype(np.float32)


# revision 25
# speedup vs baseline: 1.2133x; 1.2133x over previous
"""GATv2 (2-layer, GAT_56727928046275) on 8 TRN2 NeuronCores.

Strategy:
  - Nodes rebalanced (host LPT) into 8 cores x 49 windows of 128 dst
    nodes so per-(core,window) edge counts are nearly equal; edges
    partitioned by dst so per-node softmax + scatter-add stay local.
  - xl tables in DRAM hold |att|-prescaled, sign-permuted features plus
    precomputed "linear logit" columns (lrelu(s)=0.6s+0.4|s| so the
    per-edge logit = 0.6*(Ls+Rd) + 0.4*(sum|.|pos - sum|.|neg)); per-edge
    rows fetched with gpsimd dma_gather (512B rows for layer 0).
  - Scatter indicator generated on-chip (batched DVE is_equal against an
    iota row); the xr-expansion indicator is streamed from DRAM.
  - Layer-0 xl table computed fully on every core (no AllGather); table
    split in two DRAM tensors so block-0 gathers overlap block-1 writes.
  - BatchNorm stats via masked ones-matmul column sums + AllReduce;
    hidden activations stay resident in SBUF between phases.
"""

import numpy as np
import ml_dtypes

import concourse.bass as bass
import concourse.mybir as mybir
import concourse.tile as tile
from concourse.bacc import Bacc
from concourse.bass_utils import run_bass_kernel_spmd

# The ucode's 256B elem_size restriction applies only to transpose-mode
# gathers; relax the bass-level assert so non-transpose gathers can move
# sub-row payloads (row stride stays 256B-aligned via elem_step).
import inspect as _insp
import textwrap as _tw
_src = _tw.dedent(_insp.getsource(bass.BassGpSimd.dma_gather))
_src = _src.replace("elem_size_bytes > 0 and elem_size_bytes % 256 == 0",
                    "elem_size_bytes > 0")
_ns = dict(bass.__dict__)
exec(_src, _ns)
bass.BassGpSimd.dma_gather = _ns["dma_gather"]

BF = mybir.dt.bfloat16
F32 = mybir.dt.float32
I16 = mybir.dt.int16
OP = mybir.AluOpType
AF = mybir.ActivationFunctionType
bf16 = ml_dtypes.bfloat16

NEG_SLOPE = 0.2
BN_EPS = 1e-5


# ---------------------------------------------------------------- host prep

def _wrap16(a):
    """idx array [n] (n % 16 == 0) -> [128, n//16] int16 (16-wrap, x8 rep)."""
    a = np.asarray(a, np.int16).reshape(-1, 16).T  # [16, n/16]
    return np.tile(a, (8, 1))


def _rep(v, dt=np.float32):
    v = np.asarray(v, dt).reshape(1, -1)
    return np.ascontiguousarray(np.broadcast_to(v, (128, v.shape[1])))


def _sign_perm(att):
    """Per-head permutation putting att>=0 channels first.
    att: [H, C]. Returns perm [H*C] (flat), npos [H]."""
    H, C = att.shape
    perm = np.zeros(H * C, np.int64)
    npos = np.zeros(H, np.int64)
    for h in range(H):
        a = att[h]
        pos = np.nonzero(a >= 0)[0]
        neg = np.nonzero(a < 0)[0]
        perm[h * C:(h + 1) * C] = h * C + np.concatenate([pos, neg])
        npos[h] = len(pos)
    return perm, npos


def _balance_nodes(deg, n_cores, W):
    """LPT-assign nodes to n_cores*W bins of <=128 nodes, balancing the
    per-bin in-degree sums. Returns rowid[n] (table row per node)."""
    import heapq
    N = deg.shape[0]
    nbins = n_cores * W
    order = np.argsort(-deg, kind="stable")
    heap = [(0, b) for b in range(nbins)]
    heapq.heapify(heap)
    counts = np.zeros(nbins, np.int64)
    rowid = np.zeros(N, np.int64)
    spill = []
    for n in order:
        while True:
            load, b = heapq.heappop(heap)
            if counts[b] < 128:
                break
            spill.append((load, b))
        rowid[n] = b * 128 + counts[b]
        counts[b] += 1
        heapq.heappush(heap, (load + int(deg[n]), b))
    return rowid


def preprocess(x, edge_index, params, n_cores=8):
    """Build per-core input maps + program metadata."""
    N, IN = x.shape
    SLICE = N // n_cores
    SP = ((SLICE + 127) // 128) * 128          # padded slice rows (6272)
    W = SP // 128                              # windows per core (49)
    BLK = (n_cores // 2) * SP                  # int16 table split row (25088)

    src = np.concatenate([edge_index[0], np.arange(N, dtype=np.int64)])
    dst = np.concatenate([edge_index[1], np.arange(N, dtype=np.int64)])

    deg = np.bincount(dst, minlength=N)
    rowid = _balance_nodes(deg, n_cores, W)    # node -> table row
    row = rowid[src]                           # per-edge src table row
    drow = rowid[dst]
    core = drow // SP
    dloc = drow % SP                           # dst row within its core

    # per-core, per-window, per-block edge lists
    ecnt0 = np.zeros((n_cores, W), np.int64)
    ecnt1 = np.zeros((n_cores, W), np.int64)
    lists = [[None] * W for _ in range(n_cores)]
    for k in range(n_cores):
        m = core == k
        r, d = row[m], dloc[m]
        win = d // 128
        blk = (r >= BLK).astype(np.int64)
        order = np.lexsort((blk, win))
        r, d, win, blk = r[order], d[order], win[order], blk[order]
        for w in range(W):
            wm = win == w
            rw, dw, bw = r[wm], d[wm], blk[wm]
            b0 = bw == 0
            lists[k][w] = (rw[b0], dw[b0], rw[~b0], dw[~b0])
            ecnt0[k][w] = int(b0.sum())
            ecnt1[k][w] = int((~b0).sum())

    G0 = np.maximum(1, (ecnt0.max(0) + 127) // 128)   # [W]
    G1 = np.maximum(1, (ecnt1.max(0) + 127) // 128)
    GW = G0 + G1
    NG = int(GW.sum())

    # padded per-core arrays
    src_cols = int((GW * 8).sum())
    srcw = np.zeros((n_cores, 128, src_cols), np.int16)
    drel = np.full((n_cores, 128, NG), -1.0, np.float32)
    swin = np.zeros((n_cores, 128, NG * 128), bf16)
    for k in range(n_cores):
        sc = 0
        gc = 0
        for w in range(W):
            r0, d0, r1, d1 = lists[k][w]
            n0, n1 = len(r0), len(r1)
            s0 = np.zeros(int(G0[w]) * 128, np.int64)
            s0[:n0] = r0
            s1 = np.zeros(int(G1[w]) * 128, np.int64)
            s1[:n1] = r1 - BLK
            rl = np.full(int(GW[w]) * 128, -1.0, np.float32)
            rl[:n0] = (d0 - w * 128).astype(np.float32)
            rl[int(G0[w]) * 128:int(G0[w]) * 128 + n1] = (
                d1 - w * 128).astype(np.float32)
            srcw[k][:, sc:sc + int(G0[w]) * 8] = _wrap16(s0)
            srcw[k][:, sc + int(G0[w]) * 8:sc + int(GW[w]) * 8] = _wrap16(s1)
            drel[k][:, gc:gc + int(GW[w])] = rl.reshape(int(GW[w]), 128).T
            # node-major indicator for xr expansion: [n, g*128+e]
            sw = (np.arange(128)[:, None] == rl[None, :]).astype(bf16)
            swin[k][:, gc * 128:(gc + int(GW[w])) * 128] = sw
            sc += int(GW[w]) * 8
            gc += int(GW[w])

    p = params
    H, C = p["att0"].shape
    OUTC = p["att1"].shape[1]
    perm0, npos0 = _sign_perm(np.asarray(p["att0"]))
    perm1, npos1 = _sign_perm(np.asarray(p["att1"]))
    aab0 = np.abs(np.asarray(p["att0"]).reshape(-1)[perm0])   # [192]
    aab1 = np.abs(np.asarray(p["att1"]).reshape(-1)[perm1])   # [64]

    def sign_vec(npos, nh, Ch):
        s = np.zeros(nh * Ch, np.float32)
        for h in range(nh):
            s[h * Ch:h * Ch + int(npos[h])] = 1.0
            s[h * Ch + int(npos[h]):(h + 1) * Ch] = -1.0
        return s

    sgn0 = sign_vec(npos0, H, C)
    sgn1 = sign_vec(npos1, 1, OUTC)

    def prep_w(Wm, b, perm, aab, sgn, nh, Ch):
        """Permute+prescale weight; append nh linear-logit columns."""
        Wp = np.asarray(Wm, np.float64)[:, perm] * aab[None, :]
        bp = np.asarray(b, np.float64)[perm] * aab
        La = np.stack([(Wp[:, h * Ch:(h + 1) * Ch]
                        * sgn[h * Ch:(h + 1) * Ch][None, :]).sum(1)
                       for h in range(nh)], 1)      # [in, nh]
        Lb = np.array([(bp[h * Ch:(h + 1) * Ch]
                        * sgn[h * Ch:(h + 1) * Ch]).sum()
                       for h in range(nh)])
        return (np.concatenate([Wp, La], 1).astype(np.float32),
                np.concatenate([bp, Lb]).astype(np.float32))

    # biases of the xl tables are folded into the xr tables (z = xl+xr
    # only ever sees the sum), so xl-side tables need no bias add at all.
    zb0 = np.zeros_like(np.asarray(p["bl0"]))
    zb1 = np.zeros_like(np.asarray(p["bl1"]))
    Wl0p, _ = prep_w(p["Wl0"], zb0, perm0, aab0, sgn0, H, C)
    Wr0p, br0p = prep_w(p["Wr0"], np.asarray(p["bl0"]) + np.asarray(p["br0"]),
                        perm0, aab0, sgn0, H, C)
    Wl1p, _ = prep_w(np.asarray(p["Wl1"])[perm0, :], zb1,
                     perm1, aab1, sgn1, 1, OUTC)
    Wr1p, br1p = prep_w(np.asarray(p["Wr1"])[perm0, :],
                        np.asarray(p["bl1"]) + np.asarray(p["br1"]),
                        perm1, aab1, sgn1, 1, OUTC)

    FEAT0 = H * C            # 192
    NC0 = FEAT0 + H          # 195 computed cols layer 0
    NC1 = OUTC + 1           # 65 computed cols layer 1
    TBL0 = 256               # bf16 cols -> 512B rows
    TBL1 = 128               # bf16 cols -> 256B rows
    ELEM1 = 66               # gathered cols layer 1 (>=65, mult of 2)

    shared = {
        "Wl0p": Wl0p.astype(bf16), "Wr0p": Wr0p.astype(bf16),
        "BR0": _rep(br0p, bf16),
        "Wl1a": Wl1p[:128].astype(bf16), "Wl1b": Wl1p[128:].astype(bf16),
        "Wr1a": Wr1p[:128].astype(bf16), "Wr1b": Wr1p[128:].astype(bf16),
        "BR1": _rep(br1p),
        "RA0": _rep(1.0 / aab0), "RA1": _rep(1.0 / aab1),
        "BIAS0": _rep(np.asarray(p["bias0"])[perm0]),
        "BIAS1": _rep(np.asarray(p["bias1"])[perm1]),
        "gamma0": np.asarray(p["gamma0"])[perm0].reshape(1, -1).astype(
            np.float32),
        "beta0": np.asarray(p["beta0"])[perm0].reshape(1, -1).astype(
            np.float32),
        "gamma1": np.asarray(p["gamma1"])[perm1].reshape(1, -1).astype(
            np.float32),
        "beta1": np.asarray(p["beta1"])[perm1].reshape(1, -1).astype(
            np.float32),
        "IDENT": np.eye(128, dtype=bf16),
        "ONE1": np.ones((1, 128), np.float32),
    }
    shared["IOTA"] = np.tile(np.arange(128, dtype=bf16).reshape(1, -1),
                             (128, 1))

    xt = np.asarray(x).T  # [IN, N]
    xfull = np.zeros((IN, n_cores * SP), np.float32)
    xfull[:, :] = 0.0
    cols = rowid  # node n -> column rowid[n]
    xfull[:, cols] = xt
    xfull = xfull.astype(bf16)

    in_maps = []
    for k in range(n_cores):
        # per-core validity mask [128, W] (1 for slots holding real nodes)
        msk = np.zeros(SP, np.float32)
        kcols = (rowid >= k * SP) & (rowid < (k + 1) * SP)
        msk[rowid[kcols] - k * SP] = 1.0
        m = {"xT": xfull,
             "xTk": np.ascontiguousarray(xfull[:, k * SP:(k + 1) * SP]),
             "srcw": srcw[k], "swin": swin[k],
             "drel": drel[k].astype(bf16),
             "SMASK": np.ascontiguousarray(
                 msk.reshape(W, 128).T.astype(bf16))}
        for kk, v in shared.items():
            m[kk] = np.ascontiguousarray(v)
        in_maps.append(m)

    meta = dict(N=N, IN=IN, SLICE=SLICE, SP=SP, W=W, BLK=BLK,
                G0=[int(v) for v in G0], G1=[int(v) for v in G1],
                GW=[int(v) for v in GW], NG=NG, src_cols=src_cols,
                H=H, C=C, FEAT0=FEAT0, OUTC=OUTC, NC0=NC0, NC1=NC1,
                TBL0=TBL0, TBL1=TBL1, ELEM1=ELEM1, Gmax=int(GW.max()),
                npos0=[int(v) for v in npos0], npos1=[int(v) for v in npos1],
                n_cores=n_cores)
    return in_maps, meta, rowid, perm1


# ---------------------------------------------------------------- program

def build_program(meta):
    n_cores = meta["n_cores"]
    IN, SP, W = meta["IN"], meta["SP"], meta["W"]
    G0, G1, GW, NG = meta["G0"], meta["G1"], meta["GW"], meta["NG"]
    H, C, FEAT0 = meta["H"], meta["C"], meta["FEAT0"]
    OUTC = meta["OUTC"]
    NC0, NC1 = meta["NC0"], meta["NC1"]
    TBL0, TBL1, ELEM1 = meta["TBL0"], meta["TBL1"], meta["ELEM1"]
    BLKROW = (n_cores // 2) * SP
    HBLK = BLKROW // 128                       # 128-row tiles per table half
    NROW = n_cores * SP
    cores = list(range(n_cores))
    import os
    # >8 groups/op (>64 descs/engine) overflows the SWDGE ring with two
    # ops in flight on a queue -> device fault. Empirically 8 is the max.
    GB = int(os.environ.get("K_GB", "8"))      # gather groups per SWDGE op

    nc = Bacc("TRN2", target_bir_lowering=False, debug=False,
              num_devices=n_cores, num_swdge_queues=4)

    def dp(name, shape, dt):
        return nc.declare_dram_parameter(name, list(shape), dt,
                                         isOutput=False)

    xT = dp("xT", [IN, NROW], BF)
    xTk = dp("xTk", [IN, SP], BF)
    srcw = dp("srcw", [128, meta["src_cols"]], I16)
    swin = dp("swin", [128, NG * 128], BF)
    drel = dp("drel", [128, NG], BF)
    SMASK = dp("SMASK", [128, W], BF)
    Wl0p = dp("Wl0p", [IN, NC0], BF)
    Wr0p = dp("Wr0p", [IN, NC0], BF)
    BR0 = dp("BR0", [128, NC0], BF)
    Wl1a = dp("Wl1a", [128, NC1], BF)
    Wl1b = dp("Wl1b", [FEAT0 - 128, NC1], BF)
    Wr1a = dp("Wr1a", [128, NC1], BF)
    Wr1b = dp("Wr1b", [FEAT0 - 128, NC1], BF)
    BR1 = dp("BR1", [128, NC1], F32)
    RA0 = dp("RA0", [128, FEAT0], F32)
    RA1 = dp("RA1", [128, OUTC], F32)
    BIAS0 = dp("BIAS0", [128, FEAT0], F32)
    BIAS1 = dp("BIAS1", [128, OUTC], F32)
    gamma0 = dp("gamma0", [1, FEAT0], F32)
    beta0 = dp("beta0", [1, FEAT0], F32)
    gamma1 = dp("gamma1", [1, OUTC], F32)
    beta1 = dp("beta1", [1, OUTC], F32)
    IOTA = dp("IOTA", [128, 128], BF)
    IDENT = dp("IDENT", [128, 128], BF)
    ONE1 = dp("ONE1", [1, 128], F32)

    out = nc.declare_dram_parameter("out", [SP, OUTC], F32, isOutput=True)

    # internal DRAM
    XL0a = nc.dram_tensor("XL0a", [BLKROW, TBL0], BF)
    XL0b = nc.dram_tensor("XL0b", [NROW - BLKROW, TBL0], BF)
    XR0 = nc.dram_tensor("XR0", [SP, TBL0], BF)
    XL1s = nc.dram_tensor("XL1s", [SP, TBL1], BF)
    XR1 = nc.dram_tensor("XR1", [SP, TBL1], BF)
    XL1 = nc.dram_tensor("XL1", [NROW, TBL1], BF, addr_space="Shared")
    ST0i = nc.dram_tensor("ST0i", [1, 2 * FEAT0], F32)
    ST0o = nc.dram_tensor("ST0o", [1, 2 * FEAT0], F32, addr_space="Shared")
    ST1i = nc.dram_tensor("ST1i", [1, 2 * OUTC], F32)
    ST1o = nc.dram_tensor("ST1o", [1, 2 * OUTC], F32, addr_space="Shared")

    with tile.TileContext(nc) as tc:
        with tc.tile_pool(name="const", bufs=1) as cp, \
             tc.tile_pool(name="hp0p", bufs=1) as hp0p, \
             tc.tile_pool(name="hp1p", bufs=1) as hp1p:
            def ld(par, shape, dt):
                t = cp.tile(list(shape), dt, tag=f"c_{par.name}")
                nc.sync.dma_start(out=t[:], in_=par[:])
                return t

            c_iota = ld(IOTA, [128, 128], BF)
            c_ident = ld(IDENT, [128, 128], BF)
            c_one1 = ld(ONE1, [1, 128], F32)
            c_smask = ld(SMASK, [128, W], BF)
            c_wl0 = ld(Wl0p, [IN, NC0], BF)
            c_wr0 = ld(Wr0p, [IN, NC0], BF)
            c_br0 = ld(BR0, [128, NC0], BF)
            c_wl1a = ld(Wl1a, [128, NC1], BF)
            c_wl1b = ld(Wl1b, [FEAT0 - 128, NC1], BF)
            c_wr1a = ld(Wr1a, [128, NC1], BF)
            c_wr1b = ld(Wr1b, [FEAT0 - 128, NC1], BF)
            c_br1 = ld(BR1, [128, NC1], F32)
            c_ra0 = ld(RA0, [128, FEAT0], F32)
            c_ra1 = ld(RA1, [128, OUTC], F32)
            c_bias0 = ld(BIAS0, [128, FEAT0], F32)
            c_bias1 = ld(BIAS1, [128, OUTC], F32)
            c_g0 = ld(gamma0, [1, FEAT0], F32)
            c_b0 = ld(beta0, [1, FEAT0], F32)
            c_g1 = ld(gamma1, [1, OUTC], F32)
            c_b1 = ld(beta1, [1, OUTC], F32)
            c_src = ld(srcw, [128, meta["src_cols"]], I16)
            c_drel = ld(drel, [128, NG], BF)

            # ---------------- stage A: full xl0 table + local xr0 ------
            # xT loaded in 16-tile chunks, tables written in 8-tile
            # chunks; xl tables have no bias (folded into xr), so the
            # PSUM->SBUF move runs on the idle scalar engine.
            CH, WB = 16, 8
            with tc.tile_pool(name="sta", bufs=3) as sa, \
                 tc.tile_pool(name="sta_ps", bufs=4, space="PSUM") as sap:
                def tbl_rows(dram, trow, nt):
                    return dram[trow * 128:(trow + nt) * 128, :].rearrange(
                        "(j p) c -> p j c", p=128)

                def xl_chunk(t0, ntiles, dram, trow):
                    xt_t = sa.tile([IN, CH * 128], BF, tag="xt")
                    nc.sync.dma_start(
                        out=xt_t[:, 0:ntiles * 128],
                        in_=xT[:, t0 * 128:(t0 + ntiles) * 128])
                    for j0 in range(0, ntiles, WB):
                        nb = min(WB, ntiles - j0)
                        sb = sa.tile([128, WB, TBL0], BF, tag="sb")
                        for j in range(nb):
                            ps = sap.tile([128, 256], F32, tag="ps")
                            nc.tensor.matmul(
                                ps[:, 0:NC0],
                                xt_t[:, (j0 + j) * 128:(j0 + j + 1) * 128],
                                c_wl0[:], start=True, stop=True)
                            nc.scalar.copy(sb[:, j, 0:NC0], ps[:, 0:NC0])
                        nc.sync.dma_start(
                            out=tbl_rows(dram, trow + j0, nb),
                            in_=sb[:, 0:nb, :])

                for t0 in range(0, HBLK, CH):
                    xl_chunk(t0, min(CH, HBLK - t0), XL0a, t0)
                for t0 in range(HBLK, NROW // 128, CH):
                    xl_chunk(t0, min(CH, NROW // 128 - t0), XL0b,
                             t0 - HBLK)
                # local xr0 slice (keeps the folded bias)
                for t0 in range(0, W, WB):
                    nb = min(WB, W - t0)
                    xt_t = sa.tile([IN, CH * 128], BF, tag="xt")
                    nc.sync.dma_start(
                        out=xt_t[:, 0:nb * 128],
                        in_=xTk[:, t0 * 128:(t0 + nb) * 128])
                    sb = sa.tile([128, WB, TBL0], BF, tag="sb")
                    for j in range(nb):
                        ps = sap.tile([128, 256], F32, tag="ps")
                        nc.tensor.matmul(
                            ps[:, 0:NC0],
                            xt_t[:, j * 128:(j + 1) * 128],
                            c_wr0[:], start=True, stop=True)
                        nc.vector.tensor_add(sb[:, j, 0:NC0],
                                             ps[:, 0:NC0], c_br0[:])
                    nc.sync.dma_start(
                        out=tbl_rows(XR0, t0, nb),
                        in_=sb[:, 0:nb, :])

            # ---------------- edge pass helper -------------------------
            def edge_pass(XLa, XLb, XRt, tblc, elem, feat, ncol, nh, npos,
                          ra, biast, hp_pool, st_in, sal_eng):
                """One GATv2 edge pass; returns list of SBUF hp tiles."""
                ssum = cp.tile([1, 2 * feat], F32, tag=f"ssum{feat}")
                hp_tiles = []
                with tc.tile_pool(name="eg", bufs=3) as eg, \
                     tc.tile_pool(name="es", bufs=3) as es, \
                     tc.tile_pool(name="eps", bufs=2, space="PSUM") as eps, \
                     tc.tile_pool(name="xrp", bufs=1, space="PSUM") as xrp, \
                     tc.tile_pool(name="sps", bufs=1, space="PSUM") as sps:
                    sc = 0
                    gc = 0
                    spa = sps.tile([1, feat], F32, tag="spa")
                    spb = sps.tile([1, feat], F32, tag="spb")
                    qrr = [0]

                    def gathers(dst_tile, col0, tbl_ap, idx_col0, n_groups):
                        done = 0
                        while done < n_groups:
                            cnt = min(GB, n_groups - done)
                            nc.gpsimd.dma_gather(
                                dst_tile[:, col0 + done:col0 + done + cnt, :],
                                tbl_ap,
                                c_src[:, idx_col0 + done * 8:
                                      idx_col0 + (done + cnt) * 8],
                                num_idxs=cnt * 128, num_idxs_reg=cnt * 128,
                                elem_size=elem, elem_step=tblc,
                                queue_num=qrr[0])
                            qrr[0] = (qrr[0] + 1) % 4
                            done += cnt

                    for w in range(W):
                        g0, g1, gw = G0[w], G1[w], GW[w]
                        gl = eg.tile([128, gw, elem], BF, tag="gl")
                        gathers(gl, 0, XLa, sc, g0)
                        gathers(gl, g0, XLb, sc + g0 * 8, g1)
                        xw = eg.tile([128, elem], BF, tag="xw")
                        nc.sync.dma_start(
                            out=xw[:],
                            in_=XRt[w * 128:(w + 1) * 128, 0:elem])
                        sw = eg.tile([128, gw, 128], BF, tag="sw")
                        nc.sync.dma_start(
                            out=sw[:],
                            in_=swin[:, gc * 128:(gc + gw) * 128])
                        # on-chip scatter indicator: sal[e, g, d]=(drel==d)
                        sal = es.tile([128, gw, 128], BF, tag="sal")
                        sal_eng.tensor_tensor(
                            out=sal[:],
                            in0=c_drel[:, gc:gc + gw].unsqueeze(2)
                                .broadcast_to([128, gw, 128]),
                            in1=c_iota[:].unsqueeze(1)
                                .broadcast_to([128, gw, 128]),
                            op=OP.is_equal)
                        z = es.tile([128, gw, ncol], BF, tag="z")
                        for b0 in range(0, gw, 4):
                            nb = min(4, gw - b0)
                            xre = xrp.tile([128, 4, 512], F32, tag="xre")
                            for j in range(nb):
                                nc.tensor.matmul(
                                    xre[:, j, 0:ncol], sw[:, b0 + j, :],
                                    xw[:, 0:ncol], start=True, stop=True)
                            nc.vector.tensor_add(
                                z[:, b0:b0 + nb, :],
                                gl[:, b0:b0 + nb, 0:ncol],
                                xre[:, 0:nb, 0:ncol])
                        lp = es.tile([128, gw, nh], BF, tag="lp")
                        ln = es.tile([128, gw, nh], BF, tag="ln")
                        with nc.allow_low_precision(
                                reason="bf16 logit partial sums (<=64 "
                                       "terms, |logit|<~2)"):
                            for h in range(nh):
                                k = npos[h]
                                if k == 0:
                                    nc.vector.memset(lp[:, :, h], 0.0)
                                else:
                                    nc.vector.tensor_reduce(
                                        lp[:, :, h],
                                        z[:, :, h * C:h * C + k],
                                        axis=mybir.AxisListType.X, op=OP.add,
                                        apply_absolute_value=True)
                                if k == C:
                                    nc.vector.memset(ln[:, :, h], 0.0)
                                else:
                                    nc.vector.tensor_reduce(
                                        ln[:, :, h],
                                        z[:, :, h * C + k:(h + 1) * C],
                                        axis=mybir.AxisListType.X, op=OP.add,
                                        apply_absolute_value=True)
                        rhs = es.tile([128, gw, ncol], BF, tag="rhs")
                        lg = es.tile([128, gw, nh], BF, tag="lg")
                        nc.vector.tensor_sub(lg[:], lp[:], ln[:])
                        # logit = 0.6*(Ls+Rd) + 0.4*(lp-ln)
                        nc.vector.scalar_tensor_tensor(
                            lg[:], lg[:], 2.0 / 3.0, z[:, :, feat:feat + nh],
                            op0=OP.mult, op1=OP.add)
                        nc.scalar.activation(
                            rhs[:, :, feat:feat + nh], lg[:], AF.Exp,
                            scale=0.6)
                        pb = rhs[:, :, feat:feat + nh].unsqueeze(3)
                        nc.vector.tensor_mul(
                            rhs[:, :, 0:feat].rearrange(
                                "p g (h c) -> p g h c", h=nh),
                            gl[:, :, 0:feat].rearrange(
                                "p g (h c) -> p g h c", h=nh),
                            pb.broadcast_to([128, gw, nh, C]))
                        ps = eps.tile([128, ncol], F32, tag="acc")
                        for g in range(gw):
                            nc.tensor.matmul(
                                ps[:], sal[:, g, :], rhs[:, g, :],
                                start=(g == 0), stop=(g == gw - 1))
                        # finalize
                        dn = es.tile([128, nh], F32, tag="dn")
                        nc.vector.tensor_scalar_max(
                            dn[:], ps[:, feat:feat + nh], 1e-30)
                        rc = es.tile([128, nh], F32, tag="rc")
                        nc.vector.reciprocal(rc[:], dn[:])
                        hp = hp_pool.tile([128, feat], BF, tag=f"hp{w}")
                        hp_tiles.append(hp)
                        tmp = es.tile([128, feat], F32, tag="tmp")
                        for h in range(nh):
                            nc.vector.scalar_tensor_tensor(
                                tmp[:, h * C:(h + 1) * C],
                                ps[:, h * C:(h + 1) * C],
                                rc[:, h:h + 1],
                                ra[:, h * C:(h + 1) * C],
                                op0=OP.mult, op1=OP.mult)
                        nc.vector.tensor_add(tmp[:], tmp[:], biast[:])
                        nc.scalar.activation(hp[:], tmp[:], AF.Relu)
                        # stats
                        sq = es.tile([128, feat], BF, tag="sq")
                        nc.scalar.activation(sq[:], hp[:], AF.Square)
                        nc.tensor.matmul(spa[:], c_smask[:, w:w + 1], hp[:],
                                         start=(w == 0), stop=(w == W - 1))
                        nc.tensor.matmul(spb[:], c_smask[:, w:w + 1], sq[:],
                                         start=(w == 0), stop=(w == W - 1))
                        sc += gw * 8
                        gc += gw
                    nc.vector.tensor_copy(ssum[:, 0:feat], spa[:])
                    nc.vector.tensor_copy(ssum[:, feat:2 * feat], spb[:])
                nc.sync.dma_start(out=st_in[:, :], in_=ssum[:])
                return hp_tiles

            # ---------------- BN coeff helper --------------------------
            def bn_coeffs(st_out, feat, g_row, b_row, nodes):
                """AllReduced stats -> A_rep/B_rep [128, feat]."""
                st = cp.tile([1, 2 * feat], F32, tag=f"st{feat}")
                nc.sync.dma_start(out=st[:], in_=st_out[:, :])
                mu = cp.tile([1, feat], F32, tag=f"mu{feat}")
                var = cp.tile([1, feat], F32, tag=f"va{feat}")
                nc.vector.tensor_scalar_mul(mu[:], st[:, 0:feat], 1.0 / nodes)
                nc.vector.tensor_scalar_mul(var[:], st[:, feat:2 * feat],
                                            1.0 / nodes)
                t = cp.tile([1, feat], F32, tag=f"t{feat}")
                nc.vector.tensor_mul(t[:], mu[:], mu[:])
                nc.vector.tensor_sub(var[:], var[:], t[:])
                nc.vector.tensor_scalar_add(var[:], var[:], BN_EPS)
                nc.scalar.activation(t[:], var[:], AF.Sqrt)
                nc.vector.reciprocal(t[:], t[:])          # 1/sqrt(var+eps)
                arow = cp.tile([1, feat], F32, tag=f"ar{feat}")
                nc.vector.tensor_mul(arow[:], g_row[:], t[:])
                brow = cp.tile([1, feat], F32, tag=f"br{feat}")
                nc.vector.tensor_mul(t[:], mu[:], arow[:])
                nc.vector.tensor_sub(brow[:], b_row[:], t[:])
                with tc.tile_pool(name="bnp", bufs=2, space="PSUM") as bp:
                    pa = bp.tile([128, feat], F32, tag="pa")
                    nc.tensor.matmul(pa[:], c_one1[:], arow[:],
                                     start=True, stop=True)
                    Ar = cp.tile([128, feat], BF, tag=f"A{feat}")
                    nc.scalar.copy(Ar[:], pa[:])
                    pb2 = bp.tile([128, feat], F32, tag="pb")
                    nc.tensor.matmul(pb2[:], c_one1[:], brow[:],
                                     start=True, stop=True)
                    Br = cp.tile([128, feat], BF, tag=f"B{feat}")
                    nc.scalar.copy(Br[:], pb2[:])
                return Ar, Br

            # ================= layer 0 =================================
            hp0 = edge_pass(XL0a[:, :], XL0b[:, :], XR0, TBL0, TBL0, FEAT0,
                            NC0, H, meta["npos0"], c_ra0, c_bias0, hp0p,
                            ST0i, nc.vector)
            nc.gpsimd.collective_compute(
                "AllReduce", OP.add, replica_groups=[cores],
                ins=[ST0i[:, :]], outs=[ST0o[:, :]])
            A0, B0 = bn_coeffs(ST0o, FEAT0, c_g0, c_b0, meta["N"])

            # pass 2: BN apply + layer-1 tables (hp tiles from SBUF)
            with tc.tile_pool(name="p2", bufs=3) as p2, \
                 tc.tile_pool(name="p2ps", bufs=2, space="PSUM") as pp:
                for w in range(W):
                    hb = p2.tile([128, FEAT0], BF, tag="hb")
                    hp = hp0[w]
                    nc.vector.tensor_mul(hb[:], hp[:], A0[:])
                    nc.vector.tensor_add(hb[:], hb[:], B0[:])
                    pt0 = pp.tile([128, 128], BF, tag="pt0")
                    nc.tensor.transpose(pt0[:], hb[:, 0:128], c_ident[:])
                    pt1 = pp.tile([64, 128], BF, tag="pt1")
                    nc.tensor.transpose(pt1[:], hb[:, 128:192],
                                        c_ident[:])
                    ht0 = p2.tile([128, 128], BF, tag="ht0")
                    nc.scalar.copy(ht0[:], pt0[:])
                    ht1 = p2.tile([64, 128], BF, tag="ht1")
                    nc.scalar.copy(ht1[:], pt1[:])
                    for (wa, wb, btile, dram) in (
                            (c_wl1a, c_wl1b, None, XL1s),
                            (c_wr1a, c_wr1b, c_br1, XR1)):
                        px = pp.tile([128, NC1], F32, tag="px")
                        nc.tensor.matmul(px[:], ht0[:], wa[:],
                                         start=True, stop=False)
                        nc.tensor.matmul(px[:], ht1[:], wb[:],
                                         start=False, stop=True)
                        xs = p2.tile([128, TBL1], BF, tag="xs")
                        if btile is None:
                            nc.scalar.copy(xs[:, 0:NC1], px[:])
                        else:
                            nc.vector.tensor_add(xs[:, 0:NC1], px[:],
                                                 btile[:])
                        nc.sync.dma_start(
                            out=dram[w * 128:(w + 1) * 128, :],
                            in_=xs[:])

            nc.gpsimd.collective_compute(
                "AllGather", OP.bypass, replica_groups=[cores],
                ins=[XL1s[:, :]], outs=[XL1[:, :]])

            # ================= layer 1 =================================
            hp1 = edge_pass(XL1[0:BLKROW, 0:ELEM1], XL1[BLKROW:, 0:ELEM1],
                            XR1, TBL1, ELEM1, OUTC, NC1, 1, meta["npos1"],
                            c_ra1, c_bias1, hp1p, ST1i, nc.vector)
            nc.gpsimd.collective_compute(
                "AllReduce", OP.add, replica_groups=[cores],
                ins=[ST1i[:, :]], outs=[ST1o[:, :]])
            A1, B1 = bn_coeffs(ST1o, OUTC, c_g1, c_b1, meta["N"])

            with tc.tile_pool(name="p3", bufs=3) as p3:
                for w in range(W):
                    hp = hp1[w]
                    ob = p3.tile([128, OUTC], F32, tag="ob")
                    nc.vector.tensor_mul(ob[:], hp[:], A1[:])
                    nc.vector.tensor_add(ob[:], ob[:], B1[:])
                    nc.sync.dma_start(
                        out=out[w * 128:(w + 1) * 128, :],
                        in_=ob[:])

    nc.compile()
    return nc


# ---------------------------------------------------------------- entry

def kernel(**inputs):
    x = np.asarray(inputs["x"])
    edge_index = np.asarray(inputs["edge_index"])
    params = {k: np.asarray(v) for k, v in inputs.items()
              if k not in ("x", "edge_index")}
    n_cores = 8
    in_maps, meta, rowid, perm1 = preprocess(x, edge_index, params, n_cores)
    nc = build_program(meta)
    import os
    trace = bool(int(os.environ.get("K_TRACE", "0")))
    res = run_bass_kernel_spmd(nc, in_maps, list(range(n_cores)),
                               trace=trace)
    if trace:
        print(f"HW exec time: {res.exec_time_ns} ns", flush=True)
    outs = [res.results[k]["out"] for k in range(n_cores)]
    full = np.concatenate(outs, 0)       # [n_cores*SP, OUTC]
    inv = np.argsort(perm1)
    return np.ascontiguousarray(full[rowid][:, inv]).astype(np.float32)
